# revision 1
# baseline (speedup 1.0000x reference)
# LoftQ fused kernel for Trainium2 (Bass/Tile), 8-core data-parallel.
#
# reference:
#   W_q = (W_int - zero_point) * scale                  [out=4096, in=4096]
#   W   = W_q + (lora_B @ lora_A) * RANK**-0.5
#   y   = einsum('bsd,od->bso', x, W)                   x: [4, 2048, 4096]
#
# Strategy:
#   - Data-parallel: 8192 tokens sharded 1024/core; W replicated.
#   - y = x @ W_q.T + (x @ A.T) @ (scaling * B.T)
#     The low-rank term never materializes into W: we compute
#     u^T = A_T^T-contractions on the PE (K=128 matmuls, output directly
#     transposed), then append one K=16 tail matmul per output tile.
#   - W_int is fed to the device as packed int8 (values 0..15); dequant
#     (w*scale - zp*scale) runs on the Scalar engine as a single
#     ACTIVATE(Copy, scale, bias) per chunk, producing bf16.
#   - Main GEMM in bf16 (fp32 PSUM accumulation): per (o-chunk, t-tile),
#     32 K-tile matmuls [128x128]@[128x512] + 1 K=16 tail matmul.
#
# Host-side work is limited to sharding/layout packing (transpose + dtype
# packing); all FLOPs (dequant affine, both matmuls) run on device.

import numpy as np
import ml_dtypes

import concourse.bass as bass
import concourse.mybir as mybir
import concourse.tile as tile
from concourse import bacc
from concourse.bass import ts
from concourse.bass_utils import run_bass_kernel_spmd

P = 128
N_CORES = 8
RANK = 16
SCALING = RANK ** (-0.5)
BF16 = mybir.dt.bfloat16
F32 = mybir.dt.float32
I8 = mybir.dt.int8


def build_program(nc, T, D, O, R, scale, bias, OC=512, u_group=4):
    """Emit the per-core program.

    T: tokens per core, D: in_features, O: out_features, R: lora rank.
    scale/bias: dequant immediates (w*scale + bias), bias = -zp*scale.
    Inputs (per core):
      xtp  bf16 [P, D/P, T]      x-shard, transposed+partition-packed
      w8p  int8 [O/OC, P, D/P, OC]  W_int^T, chunk-packed (replicated)
      atp  bf16 [P, D/P, R]      lora_A^T packed (replicated)
      bts  bf16 [R, O]           lora_B^T * scaling (replicated)
    Output: y f32 [T, O]
    """
    DT, TT, NOC = D // P, T // P, O // OC
    xt = nc.dram_tensor("xtp", [P, DT, T], BF16, kind="ExternalInput")
    w8 = nc.dram_tensor("w8p", [NOC, P, DT, OC], I8, kind="ExternalInput")
    at = nc.dram_tensor("atp", [P, DT, R], BF16, kind="ExternalInput")
    bts = nc.dram_tensor("bts", [R, O], BF16, kind="ExternalInput")
    y = nc.dram_tensor("y", [T, O], F32, kind="ExternalOutput")
    y_ap = y.ap().rearrange("(tt p) o -> p tt o", p=P)

    COPY = mybir.ActivationFunctionType.Copy

    with tile.TileContext(nc) as tc:
        with (
            tc.tile_pool(name="const", bufs=1) as cpool,
            tc.tile_pool(name="w8pool", bufs=6) as w8pool,
            tc.tile_pool(name="wtpool", bufs=6) as wtpool,
            tc.tile_pool(name="outpool", bufs=4) as outpool,
            tc.tile_pool(name="psum", bufs=6, space="PSUM") as psum,
            tc.tile_pool(name="psum_u", bufs=2, space="PSUM") as psum_u,
        ):
            at_sb = cpool.tile([P, DT, R], BF16)
            nc.sync.dma_start(at_sb[:], at.ap())
            bts_sb = cpool.tile([R, O], BF16)
            nc.sync.dma_start(bts_sb[:], bts.ap())
            xt_sb = cpool.tile([P, DT, T], BF16)
            for dt in range(DT):
                nc.sync.dma_start(xt_sb[:, dt], xt.ap()[:, dt])

            # W chunks arrive/dequant in quarter-tiles so the PE can start
            # a chunk's matmuls after 1/4 of it is ready. Dequant runs on
            # the Vector engine (ACT Copy is ~5x slower per element).
            NQ = 4
            DQ = DT // NQ

            def load_dequant_chunk(oc):
                wqs = []
                for q in range(NQ):
                    w8_sb = w8pool.tile([P, DQ, OC], I8, tag="w8", name=f"w8_{oc}_{q}")
                    nc.sync.dma_start(w8_sb[:], w8.ap()[oc, :, q * DQ : (q + 1) * DQ])
                    wt_sb = wtpool.tile(
                        [P, DQ, OC], BF16, tag="wt", name=f"wt_{oc}_{q}"
                    )
                    nc.vector.tensor_scalar(
                        wt_sb[:],
                        w8_sb[:],
                        scale,
                        bias,
                        mybir.AluOpType.mult,
                        mybir.AluOpType.add,
                    )
                    wqs.append(wt_sb)
                return wqs

            def evict(ps, tt, oc):
                ob = outpool.tile([P, OC], F32, tag="ob", name=f"ob_{oc}_{tt}")
                nc.vector.tensor_copy(ob[:], ps[:])
                nc.sync.dma_start(y_ap[:, tt, ts(oc, OC)], ob[:])

            def tail_mm(ps, tt, oc):
                # K=16 low-rank tail: + u^T[:,t128]^T @ (scaling*B^T)[:, oc]
                nc.tensor.matmul(
                    ps[:],
                    lhsT=ut_sb[:, ts(tt, P)],
                    rhs=bts_sb[:, ts(oc, OC)],
                    start=False,
                    stop=True,
                )

            # u^T = (x @ A^T)^T, computed directly transposed with wide
            # (N=512) moving operands: psum[r, t512] += at[dt]^T @ xt[dt, t512]
            ut_sb = cpool.tile([R, T], BF16)
            UW = 512
            for h in range(T // UW):
                pu = psum_u.tile([R, UW], F32, tag="pu", name=f"pu_{h}")
                for dt in range(DT):
                    nc.tensor.matmul(
                        pu[:],
                        lhsT=at_sb[:, dt],
                        rhs=xt_sb[:, dt, ts(h, UW)],
                        start=(dt == 0),
                        stop=(dt == DT - 1),
                    )
                nc.scalar.activation(ut_sb[:, ts(h, UW)], pu[:], COPY)

            for oc in range(NOC):
                wqs = load_dequant_chunk(oc)
                for tt in range(TT):
                    ps = psum.tile([P, OC], F32, tag="ps", name=f"ps_{oc}_{tt}")
                    for dt in range(DT):
                        nc.tensor.matmul(
                            ps[:],
                            lhsT=xt_sb[:, dt, ts(tt, P)],
                            rhs=wqs[dt // DQ][:, dt % DQ],
                            start=(dt == 0),
                            stop=False,
                        )
                    tail_mm(ps, tt, oc)
                    evict(ps, tt, oc)
    return nc


def _pack_inputs(x, W_int, lora_A, lora_B):
    """Host-side shard + layout packing. Returns per-core input maps."""
    BS, S, D = x.shape
    O = W_int.shape[0]
    Tfull = BS * S
    T = Tfull // N_CORES
    DT = D // P
    OC = 512
    NOC = O // OC

    xb = np.asarray(x, dtype=np.float32).reshape(Tfull, D).astype(ml_dtypes.bfloat16)
    # [oc, p, dt, j] <- W_int^T[d=dt*P+p, o=oc*OC+j]
    w8p = np.ascontiguousarray(
        np.asarray(W_int, dtype=np.int32)
        .T.reshape(DT, P, NOC, OC)
        .transpose(2, 1, 0, 3)
        .astype(np.int8)
    )
    atp = np.ascontiguousarray(
        np.asarray(lora_A, dtype=np.float32)
        .T.reshape(DT, P, RANK)
        .transpose(1, 0, 2)
        .astype(ml_dtypes.bfloat16)
    )
    bts = np.ascontiguousarray(
        (np.asarray(lora_B, dtype=np.float32).T * SCALING).astype(ml_dtypes.bfloat16)
    )
    in_maps = []
    for c in range(N_CORES):
        xs = xb[c * T : (c + 1) * T]  # [T, D] bf16
        xtp = np.ascontiguousarray(xs.T.reshape(DT, P, T).transpose(1, 0, 2))
        in_maps.append({"xtp": xtp, "w8p": w8p, "atp": atp, "bts": bts})
    return in_maps, T, D, O


def _install_ntff_shim():
    """Provide antenv.axon_hooks (absent in this image) so that
    run_bass_kernel_spmd(trace=True) can capture NTFF profiles via the
    axon .so — mirrors trn_agent_boot.trn_boot's degraded-silently path.
    Only used for our own measurement runs (_trace=True)."""
    import sys as _sys
    import types as _types

    if "antenv.axon_hooks" in _sys.modules:
        return
    try:
        from trn_agent_boot.trn_boot import _ntff_profile_via_ctypes
    except ImportError:
        _sys.path.insert(0, "/root/.axon_site")
        from trn_agent_boot.trn_boot import _ntff_profile_via_ctypes

    hook = _ntff_profile_via_ctypes("/opt/axon/libaxon_pjrt.so")
    mod = _types.ModuleType("antenv.axon_hooks")
    mod._hook = hook
    mod.get_axon_ntff_profile_hook = lambda: mod._hook
    mod.set_axon_ntff_profile_hook = lambda h: setattr(mod, "_hook", h)
    _sys.modules["antenv.axon_hooks"] = mod
    import antenv as _antenv

    _antenv.axon_hooks = mod


def kernel(x, W_int, lora_A, lora_B, scale, zero_point, _trace=False, _tmpdir=None):
    if _trace:
        _install_ntff_shim()
    x = np.asarray(x)
    BS, S, D = x.shape
    s = float(np.asarray(scale))
    zp = float(np.asarray(zero_point))
    in_maps, T, D, O = _pack_inputs(x, W_int, lora_A, lora_B)

    nc = bacc.Bacc(
        "TRN2",
        target_bir_lowering=False,
        debug=False,
        num_devices=N_CORES,
    )
    build_program(nc, T, D, O, RANK, scale=s, bias=-zp * s)
    nc.compile()

    res = run_bass_kernel_spmd(
        nc,
        in_maps,
        core_ids=list(range(N_CORES)),
        trace=_trace,
        tmpdir=_tmpdir,
        trace_cores=list(range(N_CORES)) if _trace else None,
    )
    y = np.concatenate([r["y"] for r in res.results], axis=0).reshape(BS, S, O)
    if _trace:
        kernel.last_results = res
    return y


if __name__ == "__main__":
    # smoke: build-only for full shapes
    nc = bacc.Bacc("TRN2", target_bir_lowering=False, debug=False, num_devices=8)
    build_program(nc, 1024, 4096, 4096, 16, scale=0.01, bias=-0.075)
    nc.compile()
    print("build ok; instructions:", sum(len(b.instructions) for b in nc.main_func.blocks))



# revision 4
# speedup vs baseline: 1.6063x; 1.6063x over previous
# LoftQ fused kernel for Trainium2 (Bass/Tile), 8-core data-parallel, fp8.
#
# reference:
#   W_q = (W_int - zero_point) * scale                  [out=4096, in=4096]
#   W   = W_q + (lora_B @ lora_A) * RANK**-0.5
#   y   = einsum('bsd,od->bso', x, W)                   x: [4, 2048, 4096]
#
# Strategy:
#   - Data-parallel: 8192 tokens sharded 1024/core; W replicated.
#   - Decompose y = s*(x @ W_int.T) - s*zp*rowsum(x) + (x @ A.T) @ (sc*B.T)
#     W_int values 0..15 are EXACT in fp8e4m3, so the main GEMM runs as
#     fp8 x fp8 with MatmulPerfMode.DoubleRow (2 K-subtiles per
#     instruction, 0.5 cycles/row = 2x bf16 PE throughput).
#   - x is split hi/lo: xhi = f8(x), xlo = f8(16*(x - xhi)). The main GEMM
#     uses xhi only (error lands on the small quantized term). The LoRA
#     path u = x @ A_aug.T uses xhi@A + xlo@(A/16) for ~bf16 accuracy.
#   - zero point folded in by augmenting A with a ones row (rank 16->17):
#     u[:,16] = rowsum(x); bts row 16 = -zp. Tail matmul adds
#     u @ (sc*B.T/s | -zp) into the main PSUM; eviction scales by s.
#   - PSUM: 2 banks u-phase + 6 banks main loop (4 oc-chunks in flight).
#
# Host-side work is limited to sharding/layout packing (transpose + dtype
# packing); all FLOPs (both matmuls, dequant-by-linearity) run on device.

import numpy as np
import ml_dtypes

import concourse.bass as bass
import concourse.mybir as mybir
import concourse.tile as tile
from concourse import bacc
from concourse.bass import ts
from concourse.bass_utils import run_bass_kernel_spmd

P = 128
N_CORES = 8
RANK = 16
RA = RANK + 1  # augmented with ones-row for the zero-point rowsum
SCALING = RANK ** (-0.5)
BF16 = mybir.dt.bfloat16
F32 = mybir.dt.float32
F8 = mybir.dt.float8e4
OC = 512      # output-feature chunk (one PSUM bank wide)
OCG = 4       # chunks resident per group
NH = 2        # W chunk DMA'd in NH half-tiles for startup pipelining

DR = mybir.MatmulPerfMode.DoubleRow
COPY = mybir.ActivationFunctionType.Copy


def build_program(nc, T, D, O, scale):
    """Emit the per-core program.

    T: tokens per core, D: in_features, O: out_features.
    Inputs (per core):
      xhi  f8  [P, D/P, T]    f8(x) shard, transposed+partition-packed
      xlo  f8  [P, D/P, T]    f8(16*(x - xhi))
      w8p  f8  [O/OC, P, D/P, OC]  W_int^T chunk-packed (replicated)
      ath  bf16 [P, D/P, RA]  A_aug^T packed (replicated)
      atl  bf16 [P, D/P, RA]  A_aug^T/16 packed (replicated)
      bts  bf16 [RA, O]       [sc*B.T/s ; -zp] (replicated)
    Output: y bf16 [T, O]  (host casts to f32); y = scale * psum
    """
    DT, TT, NOC = D // P, T // P, O // OC
    ocg = min(OCG, NOC)
    assert DT % (2 * NH) == 0 and NOC % ocg == 0
    HDT = DT // NH       # dt per W half-tile
    NG = NOC // ocg      # oc groups
    UW = min(512, T)     # u-phase moving width

    xhi = nc.dram_tensor("xhi", [P, DT, T], F8, kind="ExternalInput")
    xlo = nc.dram_tensor("xlo", [P, DT, T], F8, kind="ExternalInput")
    w8 = nc.dram_tensor("w8p", [NOC, P, DT, OC], F8, kind="ExternalInput")
    ath = nc.dram_tensor("ath", [P, DT, RA], BF16, kind="ExternalInput")
    atl = nc.dram_tensor("atl", [P, DT, RA], BF16, kind="ExternalInput")
    bts = nc.dram_tensor("bts", [RA, O], BF16, kind="ExternalInput")
    y = nc.dram_tensor("y", [T, O], BF16, kind="ExternalOutput")
    y_ap = y.ap().rearrange("(tt p) o -> p tt o", p=P)

    with tile.TileContext(nc) as tc:
        with (
            tc.tile_pool(name="const", bufs=1) as cpool,
            tc.tile_pool(name="w8pool", bufs=13) as w8pool,
            tc.tile_pool(name="outpool", bufs=4) as outpool,
            tc.tile_pool(name="psum", bufs=6, space="PSUM") as psum,
            tc.tile_pool(name="psum_u", bufs=2, space="PSUM") as psum_u,
        ):
            # x shards first (u-phase + main stationaries), then lora consts.
            xhi_sb = cpool.tile([P, DT, T], F8)
            for dt in range(DT):
                nc.sync.dma_start(xhi_sb[:, dt], xhi.ap()[:, dt])
            ath_sb = cpool.tile([P, DT, RA], BF16)
            nc.sync.dma_start(ath_sb[:], ath.ap())
            atl_sb = cpool.tile([P, DT, RA], BF16)
            nc.sync.dma_start(atl_sb[:], atl.ap())
            bts_sb = cpool.tile([RA, O], BF16)
            nc.sync.dma_start(bts_sb[:], bts.ap())
            xlo_sb = cpool.tile([P, DT, T], F8)
            for dt in range(DT):
                nc.sync.dma_start(xlo_sb[:, dt], xlo.ap()[:, dt])

            # W half-tiles, issued in consumption order.
            wh = {}
            for g in range(NG):
                for h in range(NH):
                    for oc in range(g * ocg, (g + 1) * ocg):
                        w8_sb = w8pool.tile(
                            [P, HDT, OC], F8, tag="w8", name=f"w8_{oc}_{h}"
                        )
                        nc.sync.dma_start(
                            w8_sb[:], w8.ap()[oc, :, h * HDT : (h + 1) * HDT]
                        )
                        wh[oc, h] = w8_sb

            # u^T = (x @ A_aug^T)^T computed directly transposed:
            # psum[r, t512] += ath[dt]^T @ xhi[dt, t512]  (+ atl^T @ xlo)
            ut_sb = cpool.tile([RA, T], BF16)
            pu = []
            for hb in range(T // UW):
                p = psum_u.tile([RA, UW], F32, tag="pu", name=f"pu_{hb}")
                pu.append(p)
                for dt in range(DT):
                    nc.tensor.matmul(
                        p[:],
                        lhsT=ath_sb[:, dt],
                        rhs=xhi_sb[:, dt, ts(hb, UW)],
                        start=(dt == 0),
                        stop=False,
                    )
            for hb in range(T // UW):
                for dt in range(DT):
                    nc.tensor.matmul(
                        pu[hb][:],
                        lhsT=atl_sb[:, dt],
                        rhs=xlo_sb[:, dt, ts(hb, UW)],
                        start=False,
                        stop=(dt == DT - 1),
                    )
            for hb in range(T // UW):
                nc.scalar.activation(ut_sb[:, ts(hb, UW)], pu[hb][:], COPY)

            # Main loop: fp8 DoubleRow GEMM, lora+zp tail folded into the
            # same accumulation group, eviction scales by s.
            HD2 = HDT // 2  # dt2 pairs per half-tile
            for g in range(NG):
                for tt in range(TT):
                    ps = {}
                    for oc in range(g * ocg, (g + 1) * ocg):
                        ps[oc] = psum.tile([P, OC], F32, tag="ps", name=f"ps_{oc}_{tt}")
                        # K=17 tail opens the accumulation group
                        nc.tensor.matmul(
                            ps[oc][:],
                            lhsT=ut_sb[:, ts(tt, P)],
                            rhs=bts_sb[:, ts(oc, OC)],
                            start=True,
                            stop=False,
                        )
                    for dt2 in range(DT // 2):
                        h, l2 = dt2 // HD2, (dt2 % HD2) * 2
                        for oc in range(g * ocg, (g + 1) * ocg):
                            nc.tensor.matmul(
                                ps[oc][:],
                                lhsT=xhi_sb[:, 2 * dt2 : 2 * dt2 + 2, ts(tt, P)],
                                rhs=wh[oc, h][:, l2 : l2 + 2],
                                start=False,
                                stop=(dt2 == DT // 2 - 1),
                                perf_mode=DR,
                            )
                    for oc in range(g * ocg, (g + 1) * ocg):
                        ob = outpool.tile([P, OC], BF16, tag="ob", name=f"ob_{oc}_{tt}")
                        nc.vector.tensor_scalar(
                            ob[:], ps[oc][:], scale, None, mybir.AluOpType.mult
                        )
                        nc.sync.dma_start(y_ap[:, tt, ts(oc, OC)], ob[:])
    return nc


def _pack_inputs(x, W_int, lora_A, lora_B, scale, zero_point):
    """Host-side shard + layout packing. Returns per-core input maps."""
    F8NP = ml_dtypes.float8_e4m3
    BFNP = ml_dtypes.bfloat16
    BS, S, D = x.shape
    O = W_int.shape[0]
    Tfull = BS * S
    T = Tfull // N_CORES
    DT = D // P
    NOC = O // OC
    s = float(scale)
    zp = float(zero_point)

    def pack_x(v):  # [T, D] -> [P, DT, T]
        return np.ascontiguousarray(v.T.reshape(DT, P, T).transpose(1, 0, 2))

    xf = np.asarray(x, dtype=np.float32).reshape(Tfull, D)
    # [oc, p, dt, j] <- W_int^T[d=dt*P+p, o=oc*OC+j], exact in fp8e4m3
    w8p = np.ascontiguousarray(
        np.asarray(W_int, dtype=np.float32)
        .astype(F8NP)
        .T.reshape(DT, P, NOC, OC)
        .transpose(2, 1, 0, 3)
    )
    A_aug = np.concatenate(
        [np.asarray(lora_A, dtype=np.float32), np.ones((1, D), np.float32)], axis=0
    )  # [RA, D]
    ath = np.ascontiguousarray(
        A_aug.T.reshape(DT, P, RA).transpose(1, 0, 2).astype(BFNP)
    )
    atl = np.ascontiguousarray(
        (A_aug.T / 16.0).reshape(DT, P, RA).transpose(1, 0, 2).astype(BFNP)
    )
    bts = np.ascontiguousarray(
        np.concatenate(
            [
                np.asarray(lora_B, dtype=np.float32).T * (SCALING / s),
                np.full((1, O), -zp, np.float32),
            ],
            axis=0,
        ).astype(BFNP)
    )
    in_maps = []
    for c in range(N_CORES):
        xs = xf[c * T : (c + 1) * T]  # [T, D] f32
        xhi8 = xs.astype(F8NP)
        xlo8 = ((xs - xhi8.astype(np.float32)) * 16.0).astype(F8NP)
        in_maps.append(
            {
                "xhi": pack_x(xhi8),
                "xlo": pack_x(xlo8),
                "w8p": w8p,
                "ath": ath,
                "atl": atl,
                "bts": bts,
            }
        )
    return in_maps, T, D, O


def _install_ntff_shim():
    """Provide antenv.axon_hooks (absent in this image) so that
    run_bass_kernel_spmd(trace=True) can capture NTFF profiles via the
    axon .so — mirrors trn_agent_boot.trn_boot's degraded-silently path.
    Only used for our own measurement runs (_trace=True)."""
    import sys as _sys
    import types as _types

    if "antenv.axon_hooks" in _sys.modules:
        return
    try:
        from trn_agent_boot.trn_boot import _ntff_profile_via_ctypes
    except ImportError:
        _sys.path.insert(0, "/root/.axon_site")
        from trn_agent_boot.trn_boot import _ntff_profile_via_ctypes

    hook = _ntff_profile_via_ctypes("/opt/axon/libaxon_pjrt.so")
    mod = _types.ModuleType("antenv.axon_hooks")
    mod._hook = hook
    mod.get_axon_ntff_profile_hook = lambda: mod._hook
    mod.set_axon_ntff_profile_hook = lambda h: setattr(mod, "_hook", h)
    _sys.modules["antenv.axon_hooks"] = mod
    import antenv as _antenv

    _antenv.axon_hooks = mod


def kernel(x, W_int, lora_A, lora_B, scale, zero_point, _trace=False, _tmpdir=None):
    if _trace:
        _install_ntff_shim()
    x = np.asarray(x)
    BS, S, D = x.shape
    s = float(np.asarray(scale))
    zp = float(np.asarray(zero_point))
    in_maps, T, D, O = _pack_inputs(x, W_int, lora_A, lora_B, s, zp)

    nc = bacc.Bacc(
        "TRN2",
        target_bir_lowering=False,
        debug=False,
        num_devices=N_CORES,
    )
    build_program(nc, T, D, O, scale=s)
    nc.compile()

    res = run_bass_kernel_spmd(
        nc,
        in_maps,
        core_ids=list(range(N_CORES)),
        trace=_trace,
        tmpdir=_tmpdir,
        trace_cores=list(range(N_CORES)) if _trace else None,
    )
    y = (
        np.concatenate([np.asarray(r["y"]) for r in res.results], axis=0)
        .astype(np.float32)
        .reshape(BS, S, O)
    )
    if _trace:
        kernel.last_results = res
    return y


if __name__ == "__main__":
    # smoke: build-only for full shapes
    nc = bacc.Bacc("TRN2", target_bir_lowering=False, debug=False, num_devices=8)
    build_program(nc, 1024, 4096, 4096, scale=0.01)
    nc.compile()
    print("build ok; instructions:", sum(len(b.instructions) for b in nc.main_func.blocks))


# revision 9
# speedup vs baseline: 1.7001x; 1.0584x over previous
# LoftQ fused kernel for Trainium2 (Bass/Tile), 8-core data-parallel, fp8.
#
# reference:
#   W_q = (W_int - zero_point) * scale                  [out=4096, in=4096]
#   W   = W_q + (lora_B @ lora_A) * RANK**-0.5
#   y   = einsum('bsd,od->bso', x, W)                   x: [4, 2048, 4096]
#
# Strategy:
#   - Data-parallel: 8192 tokens sharded 1024/core; W replicated.
#   - Decompose y = s*(x @ W_int.T) - s*zp*rowsum(x) + (x @ A.T) @ (sc*B.T)
#     W_int values 0..15 are EXACT in fp8e4m3, so the main GEMM runs as
#     fp8 x fp8 with MatmulPerfMode.DoubleRow (2 K-subtiles per
#     instruction, 0.5 cycles/row = 2x bf16 PE throughput).
#   - x is split hi/lo: xhi = f8(x), xlo = f8(16*(x - xhi)). The main GEMM
#     uses xhi only (error lands on the small quantized term). The LoRA
#     path u = x @ A_aug.T uses xhi@A + xlo@(A/16) for ~bf16 accuracy.
#   - zero point folded in by augmenting A with a ones row (rank 16->17):
#     u[:,16] = rowsum(x); bts row 16 = -zp. Tail matmul adds
#     u @ (sc*B.T/s | -zp) into the main PSUM; eviction scales by s.
#   - PSUM: 2 banks u-phase + 6 banks main loop (4 oc-chunks in flight).
#
# Host-side work is limited to sharding/layout packing (transpose + dtype
# packing); all FLOPs (both matmuls, dequant-by-linearity) run on device.

import numpy as np
import ml_dtypes

import concourse.bass as bass
import concourse.mybir as mybir
import concourse.tile as tile
from concourse import bacc
from concourse.bass import ts
from concourse.bass_utils import run_bass_kernel_spmd

P = 128
N_CORES = 8
RANK = 16
RA = RANK + 1  # augmented with ones-row for the zero-point rowsum
SCALING = RANK ** (-0.5)
BF16 = mybir.dt.bfloat16
F32 = mybir.dt.float32
F8 = mybir.dt.float8e4
OC = 512      # output-feature chunk (one PSUM bank wide)
OCG = 2       # chunks resident per group
NH = 2        # W chunk DMA'd in NH half-tiles for startup pipelining

DR = mybir.MatmulPerfMode.DoubleRow
COPY = mybir.ActivationFunctionType.Copy


def build_program(nc, T, D, O, scale):
    """Emit the per-core program.

    T: tokens per core, D: in_features, O: out_features.
    Inputs (per core):
      xhi  f8  [P, D/P, T]    f8(x) shard, transposed+partition-packed
      xlo  f8  [P, D/P, T]    f8(16*(x - xhi))
      w8p  f8  [O/OC, P, D/P, OC]  W_int^T chunk-packed (replicated)
      ath  bf16 [P, D/P, RA]  A_aug^T packed (replicated)
      atl  bf16 [P, D/P, RA]  A_aug^T/16 packed (replicated)
      bts  bf16 [RA, O]       [sc*B.T/s ; -zp] (replicated)
    Output: y bf16 [T, O]  (host casts to f32); y = scale * psum
    """
    DT, TT, NOC = D // P, T // P, O // OC
    ocg = min(OCG, NOC)
    assert DT % (2 * NH) == 0 and NOC % ocg == 0
    HDT = DT // NH       # dt per W half-tile
    NG = NOC // ocg      # oc groups
    UW = min(512, T)     # u-phase moving width

    xhi = nc.dram_tensor("xhi", [P, DT, T], F8, kind="ExternalInput")
    xlo = nc.dram_tensor("xlo", [P, DT, T], F8, kind="ExternalInput")
    w8 = nc.dram_tensor("w8p", [NOC, P, DT, OC], F8, kind="ExternalInput")
    ath = nc.dram_tensor("ath", [P, DT, RA], BF16, kind="ExternalInput")
    atl = nc.dram_tensor("atl", [P, DT, RA], BF16, kind="ExternalInput")
    bts = nc.dram_tensor("bts", [RA, O], BF16, kind="ExternalInput")
    y = nc.dram_tensor("y", [T, O], BF16, kind="ExternalOutput")
    y_ap = y.ap().rearrange("(tt p) o -> p tt o", p=P)

    with tile.TileContext(nc) as tc:
        with (
            tc.tile_pool(name="const", bufs=1) as cpool,
            tc.tile_pool(name="w8pool", bufs=13) as w8pool,
            tc.tile_pool(name="outpool", bufs=4) as outpool,
            tc.tile_pool(name="psum", bufs=6, space="PSUM") as psum,
            tc.tile_pool(name="psum_u", bufs=2, space="PSUM") as psum_u,
        ):
            # Small consts first: the first u-phase matmul needs ath + the
            # first xhi block, so these must land before the x flood.
            ath_sb = cpool.tile([P, DT, RA], BF16)
            nc.sync.dma_start(ath_sb[:], ath.ap())
            atl_sb = cpool.tile([P, DT, RA], BF16)
            nc.sync.dma_start(atl_sb[:], atl.ap())
            bts_sb = cpool.tile([RA, O], BF16)
            nc.sync.dma_start(bts_sb[:], bts.ap())

            # Per-queue DMA bandwidth is ~1/16 of HBM, so wide tensors are
            # split across many dma_starts (-> many queues) to land fast.
            XB = max(1, DT // 16)  # dt per xhi/xlo DMA block
            xhi_sb = cpool.tile([P, DT, T], F8)
            for b in range(DT // XB):
                nc.sync.dma_start(
                    xhi_sb[:, b * XB : (b + 1) * XB], xhi.ap()[:, b * XB : (b + 1) * XB]
                )

            # W half-tiles, issued in consumption order; the first group's
            # first halves are split finest (needed right after u-phase).
            wh = {}
            w_order = []
            for g in range(NG):
                for h in range(NH):
                    for oc in range(g * ocg, (g + 1) * ocg):
                        nsub = 4 if (g == 0 and h == 0) else (2 if g == 0 else 1)
                        w_order.append((oc, h, nsub))
            for oc, h, nsub in w_order:
                wh[oc, h] = w8pool.tile([P, HDT, OC], F8, tag="w8", name=f"w8_{oc}_{h}")

            def dma_w(oc, h, nsub):
                sub = HDT // nsub
                for q in range(nsub):
                    nc.sync.dma_start(
                        wh[oc, h][:, q * sub : (q + 1) * sub],
                        w8.ap()[oc, :, h * HDT + q * sub : h * HDT + (q + 1) * sub],
                    )

            for oc, h, nsub in w_order[:ocg]:  # group 0 h0: between xhi and xlo
                dma_w(oc, h, nsub)
            xlo_sb = cpool.tile([P, DT, T], F8)
            for b in range(DT // XB):
                nc.sync.dma_start(
                    xlo_sb[:, b * XB : (b + 1) * XB], xlo.ap()[:, b * XB : (b + 1) * XB]
                )
            for oc, h, nsub in w_order[ocg:]:
                dma_w(oc, h, nsub)

            # u^T = (x @ A_aug^T)^T computed directly transposed:
            # psum[r, t512] += ath[dt]^T @ xhi[dt, t512]  (+ atl^T @ xlo)
            # hb loops are inner so consumption tracks the dt-ordered x DMAs
            ut_sb = cpool.tile([RA, T], BF16)
            pu = [
                psum_u.tile([RA, UW], F32, tag="pu", name=f"pu_{hb}")
                for hb in range(T // UW)
            ]
            for dt in range(DT):
                for hb in range(T // UW):
                    nc.tensor.matmul(
                        pu[hb][:],
                        lhsT=ath_sb[:, dt],
                        rhs=xhi_sb[:, dt, ts(hb, UW)],
                        start=(dt == 0),
                        stop=False,
                    )
            for dt in range(DT):
                for hb in range(T // UW):
                    nc.tensor.matmul(
                        pu[hb][:],
                        lhsT=atl_sb[:, dt],
                        rhs=xlo_sb[:, dt, ts(hb, UW)],
                        start=False,
                        stop=(dt == DT - 1),
                    )
            for hb in range(T // UW):
                nc.scalar.activation(ut_sb[:, ts(hb, UW)], pu[hb][:], COPY)

            # Main loop: fp8 DoubleRow GEMM, lora+zp tail folded into the
            # same accumulation group, eviction scales by s.
            HD2 = HDT // 2  # dt2 pairs per half-tile
            for g in range(NG):
                for tt in range(TT):
                    ps = {}
                    for oc in range(g * ocg, (g + 1) * ocg):
                        ps[oc] = psum.tile([P, OC], F32, tag="ps", name=f"ps_{oc}_{tt}")
                        # K=17 tail opens the accumulation group
                        nc.tensor.matmul(
                            ps[oc][:],
                            lhsT=ut_sb[:, ts(tt, P)],
                            rhs=bts_sb[:, ts(oc, OC)],
                            start=True,
                            stop=False,
                        )
                    for dt2 in range(DT // 2):
                        h, l2 = dt2 // HD2, (dt2 % HD2) * 2
                        for oc in range(g * ocg, (g + 1) * ocg):
                            nc.tensor.matmul(
                                ps[oc][:],
                                lhsT=xhi_sb[:, 2 * dt2 : 2 * dt2 + 2, ts(tt, P)],
                                rhs=wh[oc, h][:, l2 : l2 + 2],
                                start=False,
                                stop=(dt2 == DT // 2 - 1),
                                perf_mode=DR,
                            )
                    for oc in range(g * ocg, (g + 1) * ocg):
                        ob = outpool.tile([P, OC], BF16, tag="ob", name=f"ob_{oc}_{tt}")
                        nc.vector.tensor_scalar(
                            ob[:], ps[oc][:], scale, None, mybir.AluOpType.mult
                        )
                        nc.sync.dma_start(y_ap[:, tt, ts(oc, OC)], ob[:])
    return nc


def _pack_inputs(x, W_int, lora_A, lora_B, scale, zero_point):
    """Host-side shard + layout packing. Returns per-core input maps."""
    F8NP = ml_dtypes.float8_e4m3
    BFNP = ml_dtypes.bfloat16
    BS, S, D = x.shape
    O = W_int.shape[0]
    Tfull = BS * S
    T = Tfull // N_CORES
    DT = D // P
    NOC = O // OC
    s = float(scale)
    zp = float(zero_point)

    def pack_x(v):  # [T, D] -> [P, DT, T]
        return np.ascontiguousarray(v.T.reshape(DT, P, T).transpose(1, 0, 2))

    xf = np.asarray(x, dtype=np.float32).reshape(Tfull, D)
    # [oc, p, dt, j] <- W_int^T[d=dt*P+p, o=oc*OC+j], exact in fp8e4m3
    w8p = np.ascontiguousarray(
        np.asarray(W_int, dtype=np.float32)
        .astype(F8NP)
        .T.reshape(DT, P, NOC, OC)
        .transpose(2, 1, 0, 3)
    )
    A_aug = np.concatenate(
        [np.asarray(lora_A, dtype=np.float32), np.ones((1, D), np.float32)], axis=0
    )  # [RA, D]
    ath = np.ascontiguousarray(
        A_aug.T.reshape(DT, P, RA).transpose(1, 0, 2).astype(BFNP)
    )
    atl = np.ascontiguousarray(
        (A_aug.T / 16.0).reshape(DT, P, RA).transpose(1, 0, 2).astype(BFNP)
    )
    bts = np.ascontiguousarray(
        np.concatenate(
            [
                np.asarray(lora_B, dtype=np.float32).T * (SCALING / s),
                np.full((1, O), -zp, np.float32),
            ],
            axis=0,
        ).astype(BFNP)
    )
    in_maps = []
    for c in range(N_CORES):
        xs = xf[c * T : (c + 1) * T]  # [T, D] f32
        xhi8 = xs.astype(F8NP)
        xlo8 = ((xs - xhi8.astype(np.float32)) * 16.0).astype(F8NP)
        in_maps.append(
            {
                "xhi": pack_x(xhi8),
                "xlo": pack_x(xlo8),
                "w8p": w8p,
                "ath": ath,
                "atl": atl,
                "bts": bts,
            }
        )
    return in_maps, T, D, O


def _install_ntff_shim():
    """Provide antenv.axon_hooks (absent in this image) so that
    run_bass_kernel_spmd(trace=True) can capture NTFF profiles via the
    axon .so — mirrors trn_agent_boot.trn_boot's degraded-silently path.
    Only used for our own measurement runs (_trace=True)."""
    import sys as _sys
    import types as _types

    if "antenv.axon_hooks" in _sys.modules:
        return
    try:
        from trn_agent_boot.trn_boot import _ntff_profile_via_ctypes
    except ImportError:
        _sys.path.insert(0, "/root/.axon_site")
        from trn_agent_boot.trn_boot import _ntff_profile_via_ctypes

    hook = _ntff_profile_via_ctypes("/opt/axon/libaxon_pjrt.so")
    mod = _types.ModuleType("antenv.axon_hooks")
    mod._hook = hook
    mod.get_axon_ntff_profile_hook = lambda: mod._hook
    mod.set_axon_ntff_profile_hook = lambda h: setattr(mod, "_hook", h)
    _sys.modules["antenv.axon_hooks"] = mod
    import antenv as _antenv

    _antenv.axon_hooks = mod


def kernel(x, W_int, lora_A, lora_B, scale, zero_point, _trace=False, _tmpdir=None):
    if _trace:
        _install_ntff_shim()
    x = np.asarray(x)
    BS, S, D = x.shape
    s = float(np.asarray(scale))
    zp = float(np.asarray(zero_point))
    in_maps, T, D, O = _pack_inputs(x, W_int, lora_A, lora_B, s, zp)

    nc = bacc.Bacc(
        "TRN2",
        target_bir_lowering=False,
        debug=False,
        num_devices=N_CORES,
    )
    build_program(nc, T, D, O, scale=s)
    nc.compile()

    res = run_bass_kernel_spmd(
        nc,
        in_maps,
        core_ids=list(range(N_CORES)),
        trace=_trace,
        tmpdir=_tmpdir,
        trace_cores=list(range(N_CORES)) if _trace else None,
    )
    y = (
        np.concatenate([np.asarray(r["y"]) for r in res.results], axis=0)
        .astype(np.float32)
        .reshape(BS, S, O)
    )
    if _trace:
        kernel.last_results = res
    return y


if __name__ == "__main__":
    # smoke: build-only for full shapes
    nc = bacc.Bacc("TRN2", target_bir_lowering=False, debug=False, num_devices=8)
    build_program(nc, 1024, 4096, 4096, scale=0.01)
    nc.compile()
    print("build ok; instructions:", sum(len(b.instructions) for b in nc.main_func.blocks))


# revision 13
# speedup vs baseline: 1.7108x; 1.0063x over previous
# LoftQ fused kernel for Trainium2 (Bass/Tile), 8-core data-parallel, fp8.
#
# reference:
#   W_q = (W_int - zero_point) * scale                  [out=4096, in=4096]
#   W   = W_q + (lora_B @ lora_A) * RANK**-0.5
#   y   = einsum('bsd,od->bso', x, W)                   x: [4, 2048, 4096]
#
# Strategy:
#   - Data-parallel: 8192 tokens sharded 1024/core; W replicated.
#   - Decompose y = s*(x @ W_int.T) - s*zp*rowsum(x) + (x @ A.T) @ (sc*B.T)
#     W_int values 0..15 are EXACT in fp8e4m3, so the main GEMM runs as
#     fp8 x fp8 with MatmulPerfMode.DoubleRow (2 K-subtiles per
#     instruction, 0.5 cycles/row = 2x bf16 PE throughput).
#   - x is split hi/lo: xhi = f8(x), xlo = f8(16*(x - xhi)). The main GEMM
#     uses xhi only (error lands on the small quantized term). The LoRA
#     path u = x @ A_aug.T uses xhi@A + xlo@(A/16) for ~bf16 accuracy.
#   - zero point folded in by augmenting A with a ones row (rank 16->17):
#     u[:,16] = rowsum(x); bts row 16 = -zp. Tail matmul adds
#     u @ (sc*B.T/s | -zp) into the main PSUM; eviction scales by s.
#   - PSUM: 2 banks u-phase + 6 banks main loop (4 oc-chunks in flight).
#
# Host-side work is limited to sharding/layout packing (transpose + dtype
# packing); all FLOPs (both matmuls, dequant-by-linearity) run on device.

import numpy as np
import ml_dtypes

import concourse.bass as bass
import concourse.mybir as mybir
import concourse.tile as tile
from concourse import bacc
from concourse.bass import ts
from concourse.bass_utils import run_bass_kernel_spmd

P = 128
N_CORES = 8
RANK = 16
RA = RANK + 1  # augmented with ones-row for the zero-point rowsum
SCALING = RANK ** (-0.5)
BF16 = mybir.dt.bfloat16
F32 = mybir.dt.float32
F8 = mybir.dt.float8e4
OC = 512      # output-feature chunk (one PSUM bank wide)
OCG = 2       # chunks resident per group
NH = 2        # W chunk DMA'd in NH half-tiles for startup pipelining

DR = mybir.MatmulPerfMode.DoubleRow
COPY = mybir.ActivationFunctionType.Copy


def build_program(nc, T, D, O, scale):
    """Emit the per-core program.

    T: tokens per core, D: in_features, O: out_features.
    Inputs (per core):
      xhi  f8  [P, D/P, T]    f8(x) shard, transposed+partition-packed
      xlo  f8  [P, D/P, T]    f8(16*(x - xhi))
      w8p  f8  [O/OC, P, D/P, OC]  W_int^T chunk-packed (replicated)
      ath  bf16 [P, D/P, RA]  A_aug^T packed (replicated)
      atl  bf16 [P, D/P, RA]  A_aug^T/16 packed (replicated)
      bts  bf16 [RA, O]       [sc*B.T/s ; -zp] (replicated)
    Output: y bf16 [T, O]  (host casts to f32); y = scale * psum
    """
    DT, TT, NOC = D // P, T // P, O // OC
    ocg = min(OCG, NOC)
    assert DT % (2 * NH) == 0 and NOC % ocg == 0
    HDT = DT // NH       # dt per W half-tile
    NG = NOC // ocg      # oc groups
    UW = min(512, T)     # u-phase moving width

    xhi = nc.dram_tensor("xhi", [P, DT, T], F8, kind="ExternalInput")
    xlo = nc.dram_tensor("xlo", [P, DT, T], F8, kind="ExternalInput")
    w8 = nc.dram_tensor("w8p", [NOC, P, DT, OC], F8, kind="ExternalInput")
    ath = nc.dram_tensor("ath", [P, DT, RA], BF16, kind="ExternalInput")
    atl = nc.dram_tensor("atl", [P, DT, RA], BF16, kind="ExternalInput")
    bts = nc.dram_tensor("bts", [RA, O], BF16, kind="ExternalInput")
    y = nc.dram_tensor("y", [T, O], BF16, kind="ExternalOutput")
    y_ap = y.ap().rearrange("(tt p) o -> p tt o", p=P)

    with tile.TileContext(nc) as tc:
        with (
            tc.tile_pool(name="const", bufs=1) as cpool,
            tc.tile_pool(name="w8pool", bufs=13) as w8pool,
            tc.tile_pool(name="outpool", bufs=4) as outpool,
            tc.tile_pool(name="psum", bufs=6, space="PSUM") as psum,
            tc.tile_pool(name="psum_u", bufs=2, space="PSUM") as psum_u,
        ):
            # Small consts first: the first u-phase matmul needs ath + the
            # first xhi block, so these must land before the x flood.
            ath_sb = cpool.tile([P, DT, RA], BF16)
            nc.sync.dma_start(ath_sb[:], ath.ap())
            atl_sb = cpool.tile([P, DT, RA], BF16)
            nc.sync.dma_start(atl_sb[:], atl.ap())
            bts_sb = cpool.tile([RA, O], BF16)
            nc.sync.dma_start(bts_sb[:], bts.ap())

            # Per-queue DMA bandwidth is ~1/16 of HBM, so wide tensors are
            # split across many dma_starts (-> many queues) to land fast.
            # Leading blocks are single-dt so the u-phase can start early.
            XB = max(1, DT // 16)  # dt per xhi/xlo DMA block
            xblocks = [1] * min(4, DT) if DT >= 8 else []
            while sum(xblocks) < DT:
                xblocks.append(min(XB, DT - sum(xblocks)))
            xhi_sb = cpool.tile([P, DT, T], F8)
            pos = 0
            for b in xblocks:
                nc.sync.dma_start(
                    xhi_sb[:, pos : pos + b], xhi.ap()[:, pos : pos + b]
                )
                pos += b

            # W half-tiles, issued in consumption order; the first group's
            # first halves are split finest (needed right after u-phase).
            wh = {}
            w_order = []
            for g in range(NG):
                for h in range(NH):
                    for oc in range(g * ocg, (g + 1) * ocg):
                        nsub = 4 if (g == 0 and h == 0) else (2 if g == 0 else 1)
                        nsub = min(nsub, HDT)
                        w_order.append((oc, h, nsub))
            for oc, h, nsub in w_order:
                wh[oc, h] = w8pool.tile([P, HDT, OC], F8, tag="w8", name=f"w8_{oc}_{h}")

            def dma_w(oc, h, nsub):
                sub = HDT // nsub
                for q in range(nsub):
                    nc.sync.dma_start(
                        wh[oc, h][:, q * sub : (q + 1) * sub],
                        w8.ap()[oc, :, h * HDT + q * sub : h * HDT + (q + 1) * sub],
                    )

            for oc, h, nsub in w_order[:ocg]:  # group 0 h0: between xhi and xlo
                dma_w(oc, h, nsub)
            xlo_sb = cpool.tile([P, DT, T], F8)
            pos = 0
            for b in xblocks:
                nc.sync.dma_start(
                    xlo_sb[:, pos : pos + b], xlo.ap()[:, pos : pos + b]
                )
                pos += b
            for oc, h, nsub in w_order[ocg:]:
                dma_w(oc, h, nsub)

            # u^T = (x @ A_aug^T)^T computed directly transposed:
            # psum[r, t512] += ath[dt]^T @ xhi[dt, t512]  (+ atl^T @ xlo)
            # hb loops are inner so consumption tracks the dt-ordered x DMAs
            ut_sb = cpool.tile([RA, T], BF16)
            pu = [
                psum_u.tile([RA, UW], F32, tag="pu", name=f"pu_{hb}")
                for hb in range(T // UW)
            ]

            def u_pass(at_sb, x_sb, first):
                for dt in range(DT):
                    for hb in range(T // UW):
                        nc.tensor.matmul(
                            pu[hb][:],
                            lhsT=at_sb[:, dt],
                            rhs=x_sb[:, dt, ts(hb, UW)],
                            start=(first and dt == 0),
                            stop=(not first and dt == DT - 1),
                        )

            # Main loop: fp8 DoubleRow GEMM per (group, token-tile); the
            # lora+zp tail closes each accumulation group.
            HD2 = HDT // 2  # dt2 pairs per half-tile
            iters = [(g, tt) for g in range(NG) for tt in range(TT)]
            ps = {}

            def emit_drs(g, tt):
                for oc in range(g * ocg, (g + 1) * ocg):
                    ps[oc, tt] = psum.tile([P, OC], F32, tag="ps", name=f"ps_{oc}_{tt}")
                for dt2 in range(DT // 2):
                    h, l2 = dt2 // HD2, (dt2 % HD2) * 2
                    for oc in range(g * ocg, (g + 1) * ocg):
                        nc.tensor.matmul(
                            ps[oc, tt][:],
                            lhsT=xhi_sb[:, 2 * dt2 : 2 * dt2 + 2, ts(tt, P)],
                            rhs=wh[oc, h][:, l2 : l2 + 2],
                            start=(dt2 == 0),
                            stop=False,
                            perf_mode=DR,
                        )

            def emit_tail_evict(g, tt, last):
                for oc in range(g * ocg, (g + 1) * ocg):
                    nc.tensor.matmul(
                        ps[oc, tt][:],
                        lhsT=ut_sb[:, ts(tt, P)],
                        rhs=bts_sb[:, ts(oc, OC)],
                        start=False,
                        stop=True,
                    )
                for oc in range(g * ocg, (g + 1) * ocg):
                    ob = outpool.tile([P, OC], BF16, tag="ob", name=f"ob_{oc}_{tt}")
                    nc.vector.tensor_scalar(
                        ob[:], ps[oc, tt][:], scale, None, mybir.AluOpType.mult
                    )
                    # split the final tiles' writeback to shorten the drain
                    nsp = 4 if last else 1
                    for q in range(nsp):
                        sl = ts(oc * nsp + q, OC // nsp)
                        nc.sync.dma_start(y_ap[:, tt, sl], ob[:, ts(q, OC // nsp)])

            # u-phase hi pass, then the first main tiles (which only need
            # xhi+W), then the lo pass once xlo has landed, then the rest.
            PRE = min(3, len(iters))
            u_pass(ath_sb, xhi_sb, first=True)
            for g, tt in iters[:PRE]:
                emit_drs(g, tt)
            u_pass(atl_sb, xlo_sb, first=False)
            for hb in range(T // UW):
                nc.scalar.activation(ut_sb[:, ts(hb, UW)], pu[hb][:], COPY)
            for i, (g, tt) in enumerate(iters[:PRE]):
                emit_tail_evict(g, tt, i == len(iters) - 1)
            for i, (g, tt) in enumerate(iters[PRE:], start=PRE):
                emit_drs(g, tt)
                emit_tail_evict(g, tt, i == len(iters) - 1)
    return nc


def _pack_inputs(x, W_int, lora_A, lora_B, scale, zero_point):
    """Host-side shard + layout packing. Returns per-core input maps."""
    F8NP = ml_dtypes.float8_e4m3
    BFNP = ml_dtypes.bfloat16
    BS, S, D = x.shape
    O = W_int.shape[0]
    Tfull = BS * S
    T = Tfull // N_CORES
    DT = D // P
    NOC = O // OC
    s = float(scale)
    zp = float(zero_point)

    def pack_x(v):  # [T, D] -> [P, DT, T]
        return np.ascontiguousarray(v.T.reshape(DT, P, T).transpose(1, 0, 2))

    xf = np.asarray(x, dtype=np.float32).reshape(Tfull, D)
    # [oc, p, dt, j] <- W_int^T[d=dt*P+p, o=oc*OC+j], exact in fp8e4m3
    w8p = np.ascontiguousarray(
        np.asarray(W_int, dtype=np.float32)
        .astype(F8NP)
        .T.reshape(DT, P, NOC, OC)
        .transpose(2, 1, 0, 3)
    )
    A_aug = np.concatenate(
        [np.asarray(lora_A, dtype=np.float32), np.ones((1, D), np.float32)], axis=0
    )  # [RA, D]
    ath = np.ascontiguousarray(
        A_aug.T.reshape(DT, P, RA).transpose(1, 0, 2).astype(BFNP)
    )
    atl = np.ascontiguousarray(
        (A_aug.T / 16.0).reshape(DT, P, RA).transpose(1, 0, 2).astype(BFNP)
    )
    bts = np.ascontiguousarray(
        np.concatenate(
            [
                np.asarray(lora_B, dtype=np.float32).T * (SCALING / s),
                np.full((1, O), -zp, np.float32),
            ],
            axis=0,
        ).astype(BFNP)
    )
    in_maps = []
    for c in range(N_CORES):
        xs = xf[c * T : (c + 1) * T]  # [T, D] f32
        xhi8 = xs.astype(F8NP)
        xlo8 = ((xs - xhi8.astype(np.float32)) * 16.0).astype(F8NP)
        in_maps.append(
            {
                "xhi": pack_x(xhi8),
                "xlo": pack_x(xlo8),
                "w8p": w8p,
                "ath": ath,
                "atl": atl,
                "bts": bts,
            }
        )
    return in_maps, T, D, O


def _install_ntff_shim():
    """Provide antenv.axon_hooks (absent in this image) so that
    run_bass_kernel_spmd(trace=True) can capture NTFF profiles via the
    axon .so — mirrors trn_agent_boot.trn_boot's degraded-silently path.
    Only used for our own measurement runs (_trace=True)."""
    import sys as _sys
    import types as _types

    if "antenv.axon_hooks" in _sys.modules:
        return
    try:
        from trn_agent_boot.trn_boot import _ntff_profile_via_ctypes
    except ImportError:
        _sys.path.insert(0, "/root/.axon_site")
        from trn_agent_boot.trn_boot import _ntff_profile_via_ctypes

    hook = _ntff_profile_via_ctypes("/opt/axon/libaxon_pjrt.so")
    mod = _types.ModuleType("antenv.axon_hooks")
    mod._hook = hook
    mod.get_axon_ntff_profile_hook = lambda: mod._hook
    mod.set_axon_ntff_profile_hook = lambda h: setattr(mod, "_hook", h)
    _sys.modules["antenv.axon_hooks"] = mod
    import antenv as _antenv

    _antenv.axon_hooks = mod


def kernel(x, W_int, lora_A, lora_B, scale, zero_point, _trace=False, _tmpdir=None):
    if _trace:
        _install_ntff_shim()
    x = np.asarray(x)
    BS, S, D = x.shape
    s = float(np.asarray(scale))
    zp = float(np.asarray(zero_point))
    in_maps, T, D, O = _pack_inputs(x, W_int, lora_A, lora_B, s, zp)

    nc = bacc.Bacc(
        "TRN2",
        target_bir_lowering=False,
        debug=False,
        num_devices=N_CORES,
    )
    build_program(nc, T, D, O, scale=s)
    nc.compile()

    res = run_bass_kernel_spmd(
        nc,
        in_maps,
        core_ids=list(range(N_CORES)),
        trace=_trace,
        tmpdir=_tmpdir,
        trace_cores=list(range(N_CORES)) if _trace else None,
    )
    y = (
        np.concatenate([np.asarray(r["y"]) for r in res.results], axis=0)
        .astype(np.float32)
        .reshape(BS, S, O)
    )
    if _trace:
        kernel.last_results = res
    return y


if __name__ == "__main__":
    # smoke: build-only for full shapes
    nc = bacc.Bacc("TRN2", target_bir_lowering=False, debug=False, num_devices=8)
    build_program(nc, 1024, 4096, 4096, scale=0.01)
    nc.compile()
    print("build ok; instructions:", sum(len(b.instructions) for b in nc.main_func.blocks))


# revision 26
# speedup vs baseline: 1.7150x; 1.0024x over previous
# LoftQ fused kernel for Trainium2 (Bass/Tile), 8-core data-parallel, fp8.
#
# reference:
#   W_q = (W_int - zero_point) * scale                  [out=4096, in=4096]
#   W   = W_q + (lora_B @ lora_A) * RANK**-0.5
#   y   = einsum('bsd,od->bso', x, W)                   x: [4, 2048, 4096]
#
# Strategy:
#   - Data-parallel: 8192 tokens sharded 1024/core; W replicated.
#   - Decompose y = s*(x @ W_int.T) - s*zp*rowsum(x) + (x @ A.T) @ (sc*B.T)
#     W_int values 0..15 are EXACT in fp8e4m3, so the main GEMM runs as
#     fp8 x fp8 with MatmulPerfMode.DoubleRow (2 K-subtiles per
#     instruction, 0.5 cycles/row = 2x bf16 PE throughput).
#   - x is split hi/lo: xhi = f8(x), xlo = f8(16*(x - xhi)). The main GEMM
#     uses xhi only (error lands on the small quantized term). The LoRA
#     path u = x @ A_aug.T uses xhi@A + xlo@(A/16) for ~bf16 accuracy.
#   - zero point folded in by augmenting A with a ones row (rank 16->17):
#     u[:,16] = rowsum(x); bts row 16 = -zp. Tail matmul adds
#     u @ (sc*B.T/s | -zp) into the main PSUM; eviction scales by s.
#   - PSUM: 2 banks u-phase + 6 banks main loop (4 oc-chunks in flight).
#
# Host-side work is limited to sharding/layout packing (transpose + dtype
# packing); all FLOPs (both matmuls, dequant-by-linearity) run on device.

import numpy as np
import ml_dtypes

import concourse.bass as bass
import concourse.mybir as mybir
import concourse.tile as tile
from concourse import bacc
from concourse.bass import ts
from concourse.bass_utils import run_bass_kernel_spmd

P = 128
N_CORES = 8
RANK = 16
# augmented with a ones-row for the zero-point rowsum, zero-padded to 32:
# dual-fp8 ldweights (DoubleRow) requires the k-pair stride % 16 == 0
RA = 32
SCALING = RANK ** (-0.5)
BF16 = mybir.dt.bfloat16
F32 = mybir.dt.float32
F8 = mybir.dt.float8e4
F8E5 = mybir.dt.float8e5
OC = 512      # output-feature chunk (one PSUM bank wide)
OCG = 2       # chunks resident per group
NH = 2        # W chunk DMA'd in NH half-tiles for startup pipelining

DR = mybir.MatmulPerfMode.DoubleRow
COPY = mybir.ActivationFunctionType.Copy


def build_program(nc, T, D, O, scale):
    """Emit the per-core program.

    T: tokens per core, D: in_features, O: out_features.
    Inputs (per core):
      xhi  f8  [P, D/P, T]    f8(x) shard, transposed+partition-packed
      xlo  f8  [P, D/P, T]    f8(16*(x - xhi))
      w8p  f8  [O/OC, P, D/P, OC]  W_int^T chunk-packed (replicated)
      ath  bf16 [P, D/P, RA]  A_aug^T packed (replicated)
      atl  bf16 [P, D/P, RA]  A_aug^T/16 packed (replicated)
      bts  bf16 [RA, O]       [sc*B.T/s ; -zp] (replicated)
    Output: y bf16 [T, O]  (host casts to f32); y = scale * psum
    """
    DT, TT, NOC = D // P, T // P, O // OC
    ocg = min(OCG, NOC)
    assert DT % (2 * NH) == 0 and NOC % ocg == 0
    HDT = DT // NH       # dt per W half-tile
    NG = NOC // ocg      # oc groups
    UW = min(512, T)     # u-phase moving width

    xhi = nc.dram_tensor("xhi", [P, DT, T], F8, kind="ExternalInput")
    xlo = nc.dram_tensor("xlo", [P, DT, T], F8, kind="ExternalInput")
    w8 = nc.dram_tensor("w8p", [NOC, P, DT, OC], F8, kind="ExternalInput")
    ahi = nc.dram_tensor("ahi", [P, DT, RA], F8, kind="ExternalInput")
    alo = nc.dram_tensor("alo", [P, DT, RA], F8E5, kind="ExternalInput")
    a16 = nc.dram_tensor("a16", [P, DT, RA], F8, kind="ExternalInput")
    bts = nc.dram_tensor("bts", [RA, O], BF16, kind="ExternalInput")
    y = nc.dram_tensor("y", [T, O], BF16, kind="ExternalOutput")
    y_ap = y.ap().rearrange("(tt p) o -> p tt o", p=P)

    with tile.TileContext(nc) as tc:
        with (
            tc.tile_pool(name="const", bufs=1) as cpool,
            tc.tile_pool(name="w8pool", bufs=13) as w8pool,
            tc.tile_pool(name="outpool", bufs=4) as outpool,
            tc.tile_pool(name="psum", bufs=6, space="PSUM") as psum,
            tc.tile_pool(name="psum_u", bufs=2, space="PSUM") as psum_u,
        ):
            # First xhi block lands fastest when split across 4 queues by
            # partition range; issue it before everything else so the
            # u-phase's first matmul can fire early.
            xhi_sb = cpool.tile([P, DT, T], F8)
            for q in range(4):
                nc.sync.dma_start(
                    xhi_sb[ts(q, P // 4), 0:1], xhi.ap()[ts(q, P // 4), 0:1]
                )
            ahi_sb = cpool.tile([P, DT, RA], F8)
            nc.sync.dma_start(ahi_sb[:], ahi.ap())
            alo_sb = cpool.tile([P, DT, RA], F8E5)
            nc.sync.dma_start(alo_sb[:], alo.ap())
            a16_sb = cpool.tile([P, DT, RA], F8)
            nc.sync.dma_start(a16_sb[:], a16.ap())
            bts_sb = cpool.tile([RA, O], BF16)
            nc.sync.dma_start(bts_sb[:], bts.ap())

            # Per-queue DMA bandwidth is ~1/16 of HBM, so wide tensors are
            # split across many dma_starts (-> many queues) to land fast.
            XB = max(1, DT // 16)  # dt per xhi/xlo DMA block
            xblocks = [1, 1, 1] if DT >= 8 else []
            while sum(xblocks) < DT - 1:
                xblocks.append(min(XB, DT - 1 - sum(xblocks)))
            pos = 1
            for b in xblocks:
                nc.sync.dma_start(
                    xhi_sb[:, pos : pos + b], xhi.ap()[:, pos : pos + b]
                )
                pos += b

            # W half-tiles, issued in consumption order; the first group's
            # first halves are split finest (needed right after u-phase).
            wh = {}
            w_order = []
            for g in range(NG):
                for h in range(NH):
                    for oc in range(g * ocg, (g + 1) * ocg):
                        nsub = 4 if (g == 0 and h == 0) else (2 if g == 0 else 1)
                        nsub = min(nsub, HDT)
                        w_order.append((oc, h, nsub))
            for oc, h, nsub in w_order:
                wh[oc, h] = w8pool.tile([P, HDT, OC], F8, tag="w8", name=f"w8_{oc}_{h}")

            def dma_w(oc, h, nsub):
                sub = HDT // nsub
                for q in range(nsub):
                    nc.sync.dma_start(
                        wh[oc, h][:, q * sub : (q + 1) * sub],
                        w8.ap()[oc, :, h * HDT + q * sub : h * HDT + (q + 1) * sub],
                    )

            for oc, h, nsub in w_order[:ocg]:  # group 0 h0: between xhi and xlo
                dma_w(oc, h, nsub)
            xlo_sb = cpool.tile([P, DT, T], F8)
            pos = 0
            while pos < DT:
                b = min(XB, DT - pos)
                nc.sync.dma_start(
                    xlo_sb[:, pos : pos + b], xlo.ap()[:, pos : pos + b]
                )
                pos += b
            for oc, h, nsub in w_order[ocg:]:
                dma_w(oc, h, nsub)

            # u^T = (x @ A_aug^T)^T computed directly transposed:
            # psum[r, t512] += ath[dt]^T @ xhi[dt, t512]  (+ atl^T @ xlo)
            # u-phase as fp8 DoubleRow chains (dt2-paired, 2x bf16 rate):
            #   u = xhi@Ahi.T + xhi@Alo.T + xlo@(Ahi/16).T
            # hb loops are inner so consumption tracks the dt-ordered x DMAs
            ut_sb = cpool.tile([RA, T], BF16)
            pu = [
                psum_u.tile([RA, UW], F32, tag="pu", name=f"pu_{hb}")
                for hb in range(T // UW)
            ]

            def u_pass(at_sb, x_sb, first, last):
                for dt2 in range(DT // 2):
                    for hb in range(T // UW):
                        nc.tensor.matmul(
                            pu[hb][:],
                            lhsT=at_sb[:, 2 * dt2 : 2 * dt2 + 2],
                            rhs=x_sb[:, 2 * dt2 : 2 * dt2 + 2, ts(hb, UW)],
                            start=(first and dt2 == 0),
                            stop=(last and dt2 == DT // 2 - 1),
                            perf_mode=DR,
                        )

            # Main loop: fp8 DoubleRow GEMM per (group, token-tile); the
            # lora+zp tail closes each accumulation group.
            HD2 = HDT // 2  # dt2 pairs per half-tile
            iters = [(g, tt) for g in range(NG) for tt in range(TT)]
            ps = {}

            def emit_drs(g, tt):
                for oc in range(g * ocg, (g + 1) * ocg):
                    ps[oc, tt] = psum.tile([P, OC], F32, tag="ps", name=f"ps_{oc}_{tt}")
                for dt2 in range(DT // 2):
                    h, l2 = dt2 // HD2, (dt2 % HD2) * 2
                    for oc in range(g * ocg, (g + 1) * ocg):
                        nc.tensor.matmul(
                            ps[oc, tt][:],
                            lhsT=xhi_sb[:, 2 * dt2 : 2 * dt2 + 2, ts(tt, P)],
                            rhs=wh[oc, h][:, l2 : l2 + 2],
                            start=(dt2 == 0),
                            stop=False,
                            perf_mode=DR,
                        )

            def emit_tail_evict(g, tt, last):
                for oc in range(g * ocg, (g + 1) * ocg):
                    nc.tensor.matmul(
                        ps[oc, tt][:],
                        lhsT=ut_sb[:, ts(tt, P)],
                        rhs=bts_sb[:, ts(oc, OC)],
                        start=False,
                        stop=True,
                    )
                for oc in range(g * ocg, (g + 1) * ocg):
                    ob = outpool.tile([P, OC], BF16, tag="ob", name=f"ob_{oc}_{tt}")
                    nc.vector.tensor_scalar(
                        ob[:], ps[oc, tt][:], scale, None, mybir.AluOpType.mult
                    )
                    # split the final tiles' writeback to shorten the drain
                    nsp = 2 if last else 1
                    for q in range(nsp):
                        sl = ts(oc * nsp + q, OC // nsp)
                        nc.sync.dma_start(y_ap[:, tt, sl], ob[:, ts(q, OC // nsp)])

            # u-phase xhi chains, then the first main tiles (which only need
            # xhi+W), then the xlo chain once xlo has landed, then the rest.
            PRE = min(3, len(iters))
            u_pass(ahi_sb, xhi_sb, first=True, last=False)
            u_pass(alo_sb, xhi_sb, first=False, last=False)
            for g, tt in iters[:PRE]:
                emit_drs(g, tt)
            u_pass(a16_sb, xlo_sb, first=False, last=True)
            for hb in range(T // UW):
                nc.scalar.activation(ut_sb[:, ts(hb, UW)], pu[hb][:], COPY)
            for i, (g, tt) in enumerate(iters[:PRE]):
                emit_tail_evict(g, tt, i == len(iters) - 1)
            for i, (g, tt) in enumerate(iters[PRE:], start=PRE):
                emit_drs(g, tt)
                emit_tail_evict(g, tt, i == len(iters) - 1)
    return nc


def _pack_inputs(x, W_int, lora_A, lora_B, scale, zero_point):
    """Host-side shard + layout packing. Returns per-core input maps."""
    F8NP = ml_dtypes.float8_e4m3
    BFNP = ml_dtypes.bfloat16
    BS, S, D = x.shape
    O = W_int.shape[0]
    Tfull = BS * S
    T = Tfull // N_CORES
    DT = D // P
    NOC = O // OC
    s = float(scale)
    zp = float(zero_point)

    def pack_x(v):  # [T, D] -> [P, DT, T]
        return np.ascontiguousarray(v.T.reshape(DT, P, T).transpose(1, 0, 2))

    xf = np.asarray(x, dtype=np.float32).reshape(Tfull, D)
    # [oc, p, dt, j] <- W_int^T[d=dt*P+p, o=oc*OC+j], exact in fp8e4m3
    w8p = np.ascontiguousarray(
        np.asarray(W_int, dtype=np.float32)
        .astype(F8NP)
        .T.reshape(DT, P, NOC, OC)
        .transpose(2, 1, 0, 3)
    )
    F8E5NP = ml_dtypes.float8_e5m2
    A_aug = np.concatenate(
        [
            np.asarray(lora_A, dtype=np.float32),
            np.ones((1, D), np.float32),
            np.zeros((RA - RANK - 1, D), np.float32),
        ],
        axis=0,
    )  # [RA, D]

    def pack_a(v, dt):  # [RA, D] -> [P, DT, RA]
        return np.ascontiguousarray(v.T.reshape(DT, P, RA).transpose(1, 0, 2).astype(dt))

    A_hi = A_aug.astype(F8NP).astype(np.float32)
    ahi = pack_a(A_hi, F8NP)
    alo = pack_a(A_aug - A_hi, F8E5NP)
    a16 = pack_a(A_hi / 16.0, F8NP)
    bts = np.ascontiguousarray(
        np.concatenate(
            [
                np.asarray(lora_B, dtype=np.float32).T * (SCALING / s),
                np.full((1, O), -zp, np.float32),
                np.zeros((RA - RANK - 1, O), np.float32),
            ],
            axis=0,
        ).astype(BFNP)
    )
    in_maps = []
    for c in range(N_CORES):
        xs = xf[c * T : (c + 1) * T]  # [T, D] f32
        xhi8 = xs.astype(F8NP)
        xlo8 = ((xs - xhi8.astype(np.float32)) * 16.0).astype(F8NP)
        in_maps.append(
            {
                "xhi": pack_x(xhi8),
                "xlo": pack_x(xlo8),
                "w8p": w8p,
                "ahi": ahi,
                "alo": alo,
                "a16": a16,
                "bts": bts,
            }
        )
    return in_maps, T, D, O


def _install_ntff_shim():
    """Provide antenv.axon_hooks (absent in this image) so that
    run_bass_kernel_spmd(trace=True) can capture NTFF profiles via the
    axon .so — mirrors trn_agent_boot.trn_boot's degraded-silently path.
    Only used for our own measurement runs (_trace=True)."""
    import sys as _sys
    import types as _types

    if "antenv.axon_hooks" in _sys.modules:
        return
    try:
        from trn_agent_boot.trn_boot import _ntff_profile_via_ctypes
    except ImportError:
        _sys.path.insert(0, "/root/.axon_site")
        from trn_agent_boot.trn_boot import _ntff_profile_via_ctypes

    hook = _ntff_profile_via_ctypes("/opt/axon/libaxon_pjrt.so")
    mod = _types.ModuleType("antenv.axon_hooks")
    mod._hook = hook
    mod.get_axon_ntff_profile_hook = lambda: mod._hook
    mod.set_axon_ntff_profile_hook = lambda h: setattr(mod, "_hook", h)
    _sys.modules["antenv.axon_hooks"] = mod
    import antenv as _antenv

    _antenv.axon_hooks = mod


def kernel(x, W_int, lora_A, lora_B, scale, zero_point, _trace=False, _tmpdir=None):
    if _trace:
        _install_ntff_shim()
    x = np.asarray(x)
    BS, S, D = x.shape
    s = float(np.asarray(scale))
    zp = float(np.asarray(zero_point))
    in_maps, T, D, O = _pack_inputs(x, W_int, lora_A, lora_B, s, zp)

    nc = bacc.Bacc(
        "TRN2",
        target_bir_lowering=False,
        debug=False,
        num_devices=N_CORES,
    )
    build_program(nc, T, D, O, scale=s)
    nc.compile()

    res = run_bass_kernel_spmd(
        nc,
        in_maps,
        core_ids=list(range(N_CORES)),
        trace=_trace,
        tmpdir=_tmpdir,
        trace_cores=list(range(N_CORES)) if _trace else None,
    )
    y = (
        np.concatenate([np.asarray(r["y"]) for r in res.results], axis=0)
        .astype(np.float32)
        .reshape(BS, S, O)
    )
    if _trace:
        kernel.last_results = res
    return y


if __name__ == "__main__":
    # smoke: build-only for full shapes
    nc = bacc.Bacc("TRN2", target_bir_lowering=False, debug=False, num_devices=8)
    build_program(nc, 1024, 4096, 4096, scale=0.01)
    nc.compile()
    print("build ok; instructions:", sum(len(b.instructions) for b in nc.main_func.blocks))
